# revision 12
# baseline (speedup 1.0000x reference)
"""ConvDeepSet Trainium2 kernel (8 NeuronCores, SPMD over the T/grid axis).

Math (per reference):
  l = exp(log_l);  sx = X_c/l;  st = t/l
  K[n,t] = exp(-0.5*|sx_n - st_t|)                 (x_dim == 1)
  E[c,t] = sum_n phi[n,c] * K[n,t],  phi = [1, y_c]
  h0 = [E0, E1/E0]  -> 5x Conv1d(ksize 5, SAME) with ReLU between
  out = h5.T  (T, 8)

Sharding: T axis split across 8 cores (2048 cols each). Context points and
CNN weights are replicated, so each core computes its T-slice's pooled
signal plus a 10-column receptive-field halo locally — no collectives.
Each core's working width is W=2176 (=128*17) covering global columns
[s-12, s+2164); the conv stack consumes 2 halo columns per layer.
Global SAME-padding is honored by masking columns outside [0, 16384).

Device dtypes: fp32 for the |sx-st| computation and all PSUM accumulation;
fp16 for the kernel matrix, pooled signal and conv weights/activations
(measured end-to-end error ~1e-3 scaled-max vs the fp32 reference).
"""
import os

import numpy as np

import concourse.bass as bass
import concourse.mybir as mybir
import concourse.tile as tile
from concourse import bacc
from concourse.bass_utils import run_bass_kernel_spmd

F32 = mybir.dt.float32
F16 = mybir.dt.float16
I32 = mybir.dt.int32
AF = mybir.ActivationFunctionType
ALU = mybir.AluOpType

NCORES = 8
P = 128
T_PTS = 16384
TSH = T_PTS // NCORES          # 2048 columns per core
W = 2176                       # working width = 128*17 (12-col halo + spare)
OFF = 12                       # buffer col 0 <-> global col s-12
N_C = 2048
NCTX = N_C // P                # 16 context blocks
KSZ = 5
CH = 256                       # mid-layer channels

# conv output column ranges within the working buffer, per layer
_L_RANGES = [(4, 2068), (6, 2066), (8, 2064), (10, 2062), (12, 2060)]


def _ntiles(c0, c1, step=512):
    out = []
    c = c0
    while c < c1:
        out.append((c, min(c + step, c1)))
        c += step
    return out


_cache = {}


def _build():
    if "nc" in _cache:
        return _cache["nc"], _cache["names"]

    nc = bacc.Bacc(None, target_bir_lowering=False, debug=False)
    names = {}
    with tile.TileContext(nc) as tc:
        with tc.tile_pool(name="dram", bufs=1, space="DRAM") as dram:
            d_stb = dram.tile([P, W], F32, kind="ExternalInput")
            d_sx = dram.tile([P, 2 * NCTX], F32, kind="ExternalInput")
            d_phi = dram.tile([P, 2 * NCTX], F16, kind="ExternalInput")
            d_w0 = dram.tile([2 * KSZ, CH], F16, kind="ExternalInput")
            d_wm = [dram.tile([P, 20 * P], F16, kind="ExternalInput",
                                name=f"d_wm{i}") for i in range(3)]
            d_w4 = dram.tile([P, 80], F16, kind="ExternalInput")
            d_b = dram.tile([P, 9], F32, kind="ExternalInput")
            d_mt = dram.tile([P, 17], F32, kind="ExternalInput")
            d_me = dram.tile([P, 24], F16, kind="ExternalInput")
            d_out = dram.tile([8, TSH], F32, kind="ExternalOutput")
            d_scr32 = dram.tile([2, W], F32, kind="Internal")
            d_scr16 = dram.tile([2, W], F16, kind="Internal")
            names = dict(stb=d_stb.name, sx=d_sx.name, phi=d_phi.name,
                         w0=d_w0.name, w1=d_wm[0].name, w2=d_wm[1].name,
                         w3=d_wm[2].name, w4=d_w4.name, b=d_b.name,
                         mt=d_mt.name, me=d_me.name, out=d_out.name)

            with (
                tc.tile_pool(name="persist", bufs=1) as pp,
                tc.tile_pool(name="work", bufs=3) as wp,
                tc.tile_pool(name="kp", bufs=4) as kp,
            ):
                stb = pp.tile([P, W], F32, tag="stb")
                sx = pp.tile([P, 2 * NCTX], F32, tag="sx")
                phi = pp.tile([P, 2 * NCTX], F16, tag="phi")
                w0 = pp.tile([2 * KSZ, CH], F16, tag="w0")
                wm = [pp.tile([P, 20 * P], F16, tag=f"wm{i}",
                              name=f"wm{i}") for i in range(3)]
                w4 = pp.tile([P, 80], F16, tag="w4")
                ball = pp.tile([P, 9], F32, tag="ball")
                maskT = pp.tile([P, 17], F32, tag="maskT")
                maske = pp.tile([P, 24], F16, tag="maske")

                nc.sync.dma_start(out=stb[:, 0:W // 2], in_=d_stb[:, 0:W // 2])
                nc.sync.dma_start(out=stb[:, W // 2:], in_=d_stb[:, W // 2:])
                nc.sync.dma_start(out=sx[:], in_=d_sx[:])
                nc.sync.dma_start(out=phi[:], in_=d_phi[:])
                nc.sync.dma_start(out=maskT[:], in_=d_mt[:])
                nc.sync.dma_start(out=maske[:], in_=d_me[:])

                h0 = pp.tile([2, W], F16, tag="h0")
                ezT = pp.tile([P, 34], F32, tag="ezT")

                pool_tiles = _ntiles(0, W)

                # ---- pooling phase: K tiles + phi^T @ K accumulation ----
                with tc.tile_pool(name="ezp", bufs=1, space="PSUM") as ezp:
                    ez = ezp.tile([2, W], F32, tag="ez")
                    ACT_ABS_BLOCKS = (6, 13)
                    for b in range(NCTX):
                        da = wp.tile([P, W], F32, tag="da")
                        if b == 0:
                            kb = kp.tile([P, W], F16, tag="kb")
                            for lo, hi in ((0, W // 2), (W // 2, W)):
                                t1 = wp.tile([P, W // 2], F32, tag="t1h")
                                nc.vector.tensor_scalar(
                                    t1[:], stb[:, lo:hi], sx[:, 0:1], None,
                                    ALU.subtract, ALU.bypass)
                                nc.vector.tensor_scalar(
                                    da[:, lo:hi].bitcast(I32),
                                    t1[:].bitcast(I32),
                                    0x7FFFFFFF, None, ALU.bitwise_and,
                                    ALU.bypass)
                                nc.scalar.activation(
                                    kb[:, lo:hi], da[:, lo:hi], AF.Exp,
                                    bias=0.0, scale=-0.5)
                            for (c0, c1) in pool_tiles:
                                nc.tensor.matmul(
                                    ez[:, c0:c1], phi[:, 0:2], kb[:, c0:c1],
                                    start=True, stop=False)
                            continue
                        if b in ACT_ABS_BLOCKS:
                            # |st - sx| in one ScalarE op (bias = -sx)
                            nc.scalar.activation(
                                da[:], stb[:], AF.Abs,
                                bias=sx[:, NCTX + b:NCTX + b + 1], scale=1.0)
                        else:
                            t1 = wp.tile([P, W], F32, tag="t1")
                            nc.vector.tensor_scalar(
                                t1[:], stb[:], sx[:, b:b + 1], None,
                                ALU.subtract, ALU.bypass)
                            nc.vector.tensor_scalar(
                                da[:].bitcast(I32), t1[:].bitcast(I32),
                                0x7FFFFFFF, None, ALU.bitwise_and, ALU.bypass)
                        kb = kp.tile([P, W], F16, tag="kb")
                        nc.scalar.activation(kb[:], da[:], AF.Exp,
                                             bias=0.0, scale=-0.5)
                        for (c0, c1) in pool_tiles:
                            nc.tensor.matmul(
                                ez[:, c0:c1], phi[:, 2 * b:2 * b + 2],
                                kb[:, c0:c1],
                                start=False, stop=(b == NCTX - 1))

                    # ---- normalization: transpose rows via DRAM, divide, mask ----
                    ezrow = pp.tile([2, W], F32, tag="ezrow")
                    nc.scalar.copy(ezrow[:, 0:W // 2], ez[:, 0:W // 2])
                    nc.vector.tensor_copy(ezrow[:, W // 2:], ez[:, W // 2:])
                    nc.sync.dma_start(out=d_scr32[0:1, :], in_=ezrow[0:1, :])
                    nc.scalar.dma_start(out=d_scr32[1:2, :], in_=ezrow[1:2, :])
                    for r, eng in ((0, nc.sync), (1, nc.scalar)):
                        eng.dma_start(
                            out=ezT[:, 17 * r:17 * r + 17],
                            in_=d_scr32[r:r + 1, :].rearrange(
                                "o (p j) -> (o p) j", p=P))

                rec = pp.tile([P, 17], F32, tag="rec")
                nc.vector.reciprocal(rec[:], ezT[:, 0:17])
                val1 = pp.tile([P, 17], F32, tag="val1")
                nc.vector.tensor_tensor(val1[:], ezT[:, 17:34], rec[:],
                                        ALU.mult)
                den_h = pp.tile([P, 17], F16, tag="den_h")
                nc.vector.tensor_tensor(den_h[:], ezT[:, 0:17], maskT[:],
                                        ALU.mult)
                val_h = pp.tile([P, 17], F16, tag="val_h")
                nc.vector.tensor_tensor(val_h[:], val1[:], maskT[:], ALU.mult)
                for r, src, eng in ((0, den_h, nc.sync), (1, val_h, nc.scalar)):
                    eng.dma_start(
                        out=d_scr16[r:r + 1, :].rearrange(
                            "o (p j) -> (o p) j", p=P),
                        in_=src[:])
                nc.sync.dma_start(out=h0[:], in_=d_scr16[:])

                # PE warm-keeper: dummy matmuls bridging the norm-chain
                # window so HAM stays at K=8/8 into the conv phase
                with tc.tile_pool(name="wmp", bufs=1, space="PSUM") as wmp:
                    wps = wmp.tile([2, 512], F32, tag="wps")
                    for i in range(40):
                        nc.tensor.matmul(wps[:], phi[:, 0:2], kb[:, 0:512],
                                         start=(i == 0), stop=(i == 39))

                # weight DMAs: issued late (separate queue; not on the
                # critical stb path — they overlap the pooling phase)
                nc.gpsimd.dma_start(out=w0[:], in_=d_w0[:])
                for i in range(3):
                    nc.gpsimd.dma_start(out=wm[i][:], in_=d_wm[i][:])
                nc.gpsimd.dma_start(out=w4[:], in_=d_w4[:])
                nc.gpsimd.dma_start(out=ball[:], in_=d_b[:])

                # ---- conv stack ----
                hprev = None
                with tc.tile_pool(name="cp", bufs=4, space="PSUM") as cp:
                    # layer 0: im2col rows (k, ci) -> one K=10 matmul/tile
                    hc = pp.tile([2 * KSZ, W], F16, tag="hc")
                    for k in range(KSZ):
                        eng = nc.sync if k % 2 == 0 else nc.scalar
                        eng.dma_start(out=hc[2 * k:2 * k + 2, 2:W - 2],
                                      in_=h0[0:2, k:W - 4 + k])
                    c0_, c1_ = _L_RANGES[0]
                    hcur = [pp.tile([P, W], F16, tag="h1_0", name="h1_0"),
                            pp.tile([P, W], F16, tag="h1_1", name="h1_1")]
                    for (t0, t1_) in _ntiles(c0_, c1_):
                        n = t1_ - t0
                        for m in range(2):
                            ps = cp.tile([P, 512], F32, tag="cps")
                            nc.tensor.matmul(
                                ps[:, :n], w0[:, P * m:P * m + P],
                                hc[:, t0:t1_], start=True, stop=True)
                            nc.scalar.activation(
                                hcur[m][:, t0:t1_], ps[:, :n], AF.Relu,
                                bias=ball[:, 0 + m:1 + m], scale=1.0)
                    _mask_edges(nc, hcur, maske, 0)
                    hprev = hcur

                    # layers 1..3: C 256->256
                    for li in range(3):
                        c0_, c1_ = _L_RANGES[li + 1]
                        hcur = [
                            pp.tile([P, W], F16, tag=f"h{li+2}_0",
                                    name=f"h{li+2}_0"),
                            pp.tile([P, W], F16, tag=f"h{li+2}_1",
                                    name=f"h{li+2}_1")]
                        for (t0, t1_) in _ntiles(c0_, c1_):
                            n = t1_ - t0
                            for m in range(2):
                                ps = cp.tile([P, 512], F32, tag="cps")
                                idx = 0
                                for ci in range(2):
                                    for k in range(KSZ):
                                        nc.tensor.matmul(
                                            ps[:, :n],
                                            wm[li][:, P * (10 * m + 5 * ci + k):
                                                   P * (10 * m + 5 * ci + k) + P],
                                            hprev[ci][:, t0 + k - 2:t1_ + k - 2],
                                            start=(idx == 0), stop=(idx == 9))
                                        idx += 1
                                nc.scalar.activation(
                                    hcur[m][:, t0:t1_], ps[:, :n], AF.Relu,
                                    bias=ball[:, 2 * (li + 1) + m:
                                              2 * (li + 1) + m + 1], scale=1.0)
                        _mask_edges(nc, hcur, maske, li + 1)
                        hprev = hcur

                    # layer 4: C_out=8, bias only (no relu)
                    out5 = pp.tile([8, TSH], F32, tag="out5")
                    c0_, c1_ = _L_RANGES[4]
                    for (t0, t1_) in _ntiles(c0_, c1_):
                        n = t1_ - t0
                        ps = cp.tile([8, 512], F32, tag="cps5")
                        idx = 0
                        for ci in range(2):
                            for k in range(KSZ):
                                nc.tensor.matmul(
                                    ps[:, :n],
                                    w4[:, 8 * (5 * ci + k):8 * (5 * ci + k) + 8],
                                    hprev[ci][:, t0 + k - 2:t1_ + k - 2],
                                    start=(idx == 0), stop=(idx == 9))
                                idx += 1
                        nc.scalar.activation(
                            out5[:, t0 - OFF:t1_ - OFF], ps[:, :n],
                            AF.Identity, bias=ball[0:8, 8:9], scale=1.0)

                    nc.sync.dma_start(out=d_out[:], in_=out5[:])

    nc.compile()
    _cache["nc"] = nc
    _cache["names"] = names
    return nc, names


def _mask_edges(nc, hcur, maske, layer):
    o = _L_RANGES[layer][0]
    lw = OFF - o
    for m in range(2):
        nc.vector.tensor_tensor(hcur[m][:, o:OFF], hcur[m][:, o:OFF],
                                maske[:, o:OFF], ALU.mult)
        nc.vector.tensor_tensor(hcur[m][:, 2060:2060 + lw],
                                hcur[m][:, 2060:2060 + lw],
                                maske[:, 12:12 + lw], ALU.mult)


LAST_EXEC_NS = None


def kernel(X_c, y_c, t, log_l, conv_weights, conv_biases):
    global LAST_EXEC_NS
    X_c = np.asarray(X_c, dtype=np.float32)
    y_c = np.asarray(y_c, dtype=np.float32)
    t = np.asarray(t, dtype=np.float32)
    log_l = np.asarray(log_l, dtype=np.float32)
    Ws = [np.asarray(w, dtype=np.float32) for w in conv_weights]
    bs = [np.asarray(b, dtype=np.float32) for b in conv_biases]

    l = float(np.exp(log_l[0]))
    sx = (X_c[:, 0] / l).astype(np.float32)           # (2048,)
    st = (t[:, 0] / l).astype(np.float32)             # (16384,)

    st_pad = np.zeros(OFF + T_PTS + (W + 128), dtype=np.float32)
    st_pad[OFF:OFF + T_PTS] = st

    sx_pm = np.ascontiguousarray(sx.reshape(NCTX, P).T)   # (128, 16)
    sx_col = np.concatenate([sx_pm, -sx_pm], axis=1).astype(np.float32)

    phiT = np.zeros((P, 2 * NCTX), dtype=np.float16)
    for b in range(NCTX):
        phiT[:, 2 * b] = 1.0
        phiT[:, 2 * b + 1] = y_c[P * b:P * b + P].astype(np.float16)

    w0T = np.ascontiguousarray(
        Ws[0].transpose(2, 1, 0).reshape(2 * KSZ, CH)).astype(np.float16)

    wmT = []
    for li in range(3):
        Wl = Ws[li + 1]
        arr = np.zeros((P, 20 * P), dtype=np.float16)
        for m in range(2):
            for ci in range(2):
                for k in range(KSZ):
                    idx = 10 * m + 5 * ci + k
                    arr[:, P * idx:P * idx + P] = (
                        Wl[P * m:P * m + P, P * ci:P * ci + P, k].T
                        .astype(np.float16))
        wmT.append(arr)

    w4T = np.zeros((P, 80), dtype=np.float16)
    for ci in range(2):
        for k in range(KSZ):
            idx = 5 * ci + k
            w4T[:, 8 * idx:8 * idx + 8] = (
                Ws[4][:, P * ci:P * ci + P, k].T.astype(np.float16))

    b_all = np.zeros((P, 9), dtype=np.float32)
    for li in range(4):
        for m in range(2):
            b_all[:, 2 * li + m] = bs[li][P * m:P * m + P]
    b_all[0:8, 8] = bs[4]

    nc, nm = _build()

    in_maps = []
    for c in range(NCORES):
        s = c * TSH
        stb = np.ascontiguousarray(
            np.broadcast_to(st_pad[s:s + W][None, :], (P, W))).astype(np.float32)

        g = (s - OFF) + np.arange(W)          # global col of each buffer col
        valid = (g >= 0) & (g < T_PTS)
        maskT = np.ascontiguousarray(
            valid.astype(np.float32).reshape(P, 17))
        maske = np.zeros((P, 24), dtype=np.float16)
        maske[:, 0:12] = valid[0:12][None, :]
        maske[:, 12:24] = valid[2060:2072][None, :]

        in_maps.append({
            nm["stb"]: stb, nm["sx"]: sx_col, nm["phi"]: phiT,
            nm["w0"]: w0T, nm["w1"]: wmT[0], nm["w2"]: wmT[1],
            nm["w3"]: wmT[2], nm["w4"]: w4T, nm["b"]: b_all,
            nm["mt"]: maskT, nm["me"]: maske,
        })

    trace = os.environ.get("KERNEL_TRACE", "0") == "1"
    res = run_bass_kernel_spmd(nc, in_maps, list(range(NCORES)), trace=trace)
    LAST_EXEC_NS = res.exec_time_ns

    out = np.empty((T_PTS, 8), dtype=np.float32)
    for c in range(NCORES):
        out[c * TSH:(c + 1) * TSH, :] = res.results[c][nm["out"]].T
    return out


# revision 13
# speedup vs baseline: 1.0267x; 1.0267x over previous
"""ConvDeepSet Trainium2 kernel (8 NeuronCores, SPMD over the T/grid axis).

Math (per reference):
  l = exp(log_l);  sx = X_c/l;  st = t/l
  K[n,t] = exp(-0.5*|sx_n - st_t|)                 (x_dim == 1)
  E[c,t] = sum_n phi[n,c] * K[n,t],  phi = [1, y_c]
  h0 = [E0, E1/E0]  -> 5x Conv1d(ksize 5, SAME) with ReLU between
  out = h5.T  (T, 8)

Sharding: T axis split across 8 cores (2048 cols each). Context points and
CNN weights are replicated, so each core computes its T-slice's pooled
signal plus a 10-column receptive-field halo locally — no collectives.
Each core's working width is W=2176 (=128*17) covering global columns
[s-12, s+2164); the conv stack consumes 2 halo columns per layer.
Global SAME-padding is honored by masking columns outside [0, 16384).

Device dtypes: fp32 for the |sx-st| computation and all PSUM accumulation;
fp16 for the kernel matrix, pooled signal and conv weights/activations
(measured end-to-end error ~1e-3 scaled-max vs the fp32 reference).
"""
import os

import numpy as np

import concourse.bass as bass
import concourse.mybir as mybir
import concourse.tile as tile
from concourse import bacc
from concourse.bass_utils import run_bass_kernel_spmd

F32 = mybir.dt.float32
F16 = mybir.dt.float16
I32 = mybir.dt.int32
AF = mybir.ActivationFunctionType
ALU = mybir.AluOpType

NCORES = 8
P = 128
T_PTS = 16384
TSH = T_PTS // NCORES          # 2048 columns per core
W = 2176                       # working width = 128*17 (12-col halo + spare)
OFF = 12                       # buffer col 0 <-> global col s-12
N_C = 2048
NCTX = N_C // P                # 16 context blocks
KSZ = 5
CH = 256                       # mid-layer channels

# conv output column ranges within the working buffer, per layer
_L_RANGES = [(4, 2068), (6, 2066), (8, 2064), (10, 2062), (12, 2060)]


def _ntiles(c0, c1, step=512):
    out = []
    c = c0
    while c < c1:
        out.append((c, min(c + step, c1)))
        c += step
    return out


_cache = {}


def _build():
    if "nc" in _cache:
        return _cache["nc"], _cache["names"]

    nc = bacc.Bacc(None, target_bir_lowering=False, debug=False)
    names = {}
    with tile.TileContext(nc) as tc:
        with tc.tile_pool(name="dram", bufs=1, space="DRAM") as dram:
            d_stb = dram.tile([P, W], F32, kind="ExternalInput")
            d_sx = dram.tile([P, 2 * NCTX], F32, kind="ExternalInput")
            d_phi = dram.tile([P, 2 * NCTX], F16, kind="ExternalInput")
            d_w0 = dram.tile([2 * KSZ, CH], F16, kind="ExternalInput")
            d_wm = [dram.tile([P, 20 * P], F16, kind="ExternalInput",
                                name=f"d_wm{i}") for i in range(3)]
            d_w4 = dram.tile([P, 80], F16, kind="ExternalInput")
            d_b = dram.tile([P, 9], F32, kind="ExternalInput")
            d_mt = dram.tile([P, 17], F32, kind="ExternalInput")
            d_me = dram.tile([P, 24], F16, kind="ExternalInput")
            d_out = dram.tile([8, TSH], F32, kind="ExternalOutput")
            d_scr32 = dram.tile([2, W], F32, kind="Internal")
            d_scr16 = dram.tile([2, W], F16, kind="Internal")
            names = dict(stb=d_stb.name, sx=d_sx.name, phi=d_phi.name,
                         w0=d_w0.name, w1=d_wm[0].name, w2=d_wm[1].name,
                         w3=d_wm[2].name, w4=d_w4.name, b=d_b.name,
                         mt=d_mt.name, me=d_me.name, out=d_out.name)

            with (
                tc.tile_pool(name="persist", bufs=1) as pp,
                tc.tile_pool(name="work", bufs=3) as wp,
                tc.tile_pool(name="kp", bufs=4) as kp,
            ):
                stb = pp.tile([P, W], F32, tag="stb")
                sx = pp.tile([P, 2 * NCTX], F32, tag="sx")
                phi = pp.tile([P, 2 * NCTX], F16, tag="phi")
                w0 = pp.tile([2 * KSZ, CH], F16, tag="w0")
                wm = [pp.tile([P, 20 * P], F16, tag=f"wm{i}",
                              name=f"wm{i}") for i in range(3)]
                w4 = pp.tile([P, 80], F16, tag="w4")
                ball = pp.tile([P, 9], F32, tag="ball")
                maskT = pp.tile([P, 17], F32, tag="maskT")
                maske = pp.tile([P, 24], F16, tag="maske")

                nc.sync.dma_start(out=stb[:], in_=d_stb[:])
                nc.sync.dma_start(out=sx[:], in_=d_sx[:])
                nc.sync.dma_start(out=phi[:], in_=d_phi[:])
                nc.sync.dma_start(out=maskT[:], in_=d_mt[:])
                nc.sync.dma_start(out=maske[:], in_=d_me[:])

                h0 = pp.tile([2, W], F16, tag="h0")
                ezT = pp.tile([P, 34], F32, tag="ezT")

                pool_tiles = _ntiles(0, W)

                # ---- pooling phase: K tiles + phi^T @ K accumulation ----
                with tc.tile_pool(name="ezp", bufs=1, space="PSUM") as ezp:
                    ez = ezp.tile([2, W], F32, tag="ez")
                    ACT_ABS_BLOCKS = (6, 13)
                    for b in range(NCTX):
                        da = wp.tile([P, W], F32, tag="da")
                        if b in ACT_ABS_BLOCKS:
                            # |st - sx| in one ScalarE op (bias = -sx)
                            nc.scalar.activation(
                                da[:], stb[:], AF.Abs,
                                bias=sx[:, NCTX + b:NCTX + b + 1], scale=1.0)
                        else:
                            t1 = wp.tile([P, W], F32, tag="t1")
                            nc.vector.tensor_scalar(
                                t1[:], stb[:], sx[:, b:b + 1], None,
                                ALU.subtract, ALU.bypass)
                            nc.vector.tensor_scalar(
                                da[:].bitcast(I32), t1[:].bitcast(I32),
                                0x7FFFFFFF, None, ALU.bitwise_and, ALU.bypass)
                        kb = kp.tile([P, W], F16, tag="kb")
                        nc.scalar.activation(kb[:], da[:], AF.Exp,
                                             bias=0.0, scale=-0.5)
                        for (c0, c1) in pool_tiles:
                            nc.tensor.matmul(
                                ez[:, c0:c1], phi[:, 2 * b:2 * b + 2],
                                kb[:, c0:c1],
                                start=(b == 0), stop=(b == NCTX - 1))

                    # ---- normalization: transpose rows via DRAM, divide, mask ----
                    ezrow = pp.tile([2, W], F32, tag="ezrow")
                    nc.scalar.copy(ezrow[:, 0:W // 2], ez[:, 0:W // 2])
                    nc.vector.tensor_copy(ezrow[:, W // 2:], ez[:, W // 2:])
                    nc.sync.dma_start(out=d_scr32[0:1, :], in_=ezrow[0:1, :])
                    nc.scalar.dma_start(out=d_scr32[1:2, :], in_=ezrow[1:2, :])
                    for r, eng in ((0, nc.sync), (1, nc.scalar)):
                        eng.dma_start(
                            out=ezT[:, 17 * r:17 * r + 17],
                            in_=d_scr32[r:r + 1, :].rearrange(
                                "o (p j) -> (o p) j", p=P))

                rec = pp.tile([P, 17], F32, tag="rec")
                nc.vector.reciprocal(rec[:], ezT[:, 0:17])
                val1 = pp.tile([P, 17], F32, tag="val1")
                nc.vector.tensor_tensor(val1[:], ezT[:, 17:34], rec[:],
                                        ALU.mult)
                den_h = pp.tile([P, 17], F16, tag="den_h")
                nc.vector.tensor_tensor(den_h[:], ezT[:, 0:17], maskT[:],
                                        ALU.mult)
                val_h = pp.tile([P, 17], F16, tag="val_h")
                nc.vector.tensor_tensor(val_h[:], val1[:], maskT[:], ALU.mult)
                for r, src, eng in ((0, den_h, nc.sync), (1, val_h, nc.scalar)):
                    eng.dma_start(
                        out=d_scr16[r:r + 1, :].rearrange(
                            "o (p j) -> (o p) j", p=P),
                        in_=src[:])
                nc.sync.dma_start(out=h0[:], in_=d_scr16[:])

                # PE warm-keeper: dummy matmuls bridging the norm-chain
                # window so HAM stays at K=8/8 into the conv phase
                with tc.tile_pool(name="wmp", bufs=1, space="PSUM") as wmp:
                    wps = wmp.tile([2, 512], F32, tag="wps")
                    for i in range(26):
                        nc.tensor.matmul(wps[:], phi[:, 0:2], kb[:, 0:512],
                                         start=(i == 0), stop=(i == 25))

                # weight DMAs: issued late (separate queue; not on the
                # critical stb path — they overlap the pooling phase)
                nc.gpsimd.dma_start(out=w0[:], in_=d_w0[:])
                for i in range(3):
                    nc.gpsimd.dma_start(out=wm[i][:], in_=d_wm[i][:])
                nc.gpsimd.dma_start(out=w4[:], in_=d_w4[:])
                nc.gpsimd.dma_start(out=ball[:], in_=d_b[:])

                # ---- conv stack ----
                hprev = None
                with tc.tile_pool(name="cp", bufs=4, space="PSUM") as cp:
                    # layer 0: im2col rows (k, ci) -> one K=10 matmul/tile
                    hc = pp.tile([2 * KSZ, W], F16, tag="hc")
                    for k in range(KSZ):
                        eng = nc.sync if k % 2 == 0 else nc.scalar
                        eng.dma_start(out=hc[2 * k:2 * k + 2, 2:W - 2],
                                      in_=h0[0:2, k:W - 4 + k])
                    c0_, c1_ = _L_RANGES[0]
                    hcur = [pp.tile([P, W], F16, tag="h1_0", name="h1_0"),
                            pp.tile([P, W], F16, tag="h1_1", name="h1_1")]
                    for (t0, t1_) in _ntiles(c0_, c1_):
                        n = t1_ - t0
                        for m in range(2):
                            ps = cp.tile([P, 512], F32, tag="cps")
                            nc.tensor.matmul(
                                ps[:, :n], w0[:, P * m:P * m + P],
                                hc[:, t0:t1_], start=True, stop=True)
                            nc.scalar.activation(
                                hcur[m][:, t0:t1_], ps[:, :n], AF.Relu,
                                bias=ball[:, 0 + m:1 + m], scale=1.0)
                    _mask_edges(nc, hcur, maske, 0)
                    hprev = hcur

                    # layers 1..3: C 256->256
                    for li in range(3):
                        c0_, c1_ = _L_RANGES[li + 1]
                        hcur = [
                            pp.tile([P, W], F16, tag=f"h{li+2}_0",
                                    name=f"h{li+2}_0"),
                            pp.tile([P, W], F16, tag=f"h{li+2}_1",
                                    name=f"h{li+2}_1")]
                        for (t0, t1_) in _ntiles(c0_, c1_):
                            n = t1_ - t0
                            for m in range(2):
                                ps = cp.tile([P, 512], F32, tag="cps")
                                idx = 0
                                for ci in range(2):
                                    for k in range(KSZ):
                                        nc.tensor.matmul(
                                            ps[:, :n],
                                            wm[li][:, P * (10 * m + 5 * ci + k):
                                                   P * (10 * m + 5 * ci + k) + P],
                                            hprev[ci][:, t0 + k - 2:t1_ + k - 2],
                                            start=(idx == 0), stop=(idx == 9))
                                        idx += 1
                                nc.scalar.activation(
                                    hcur[m][:, t0:t1_], ps[:, :n], AF.Relu,
                                    bias=ball[:, 2 * (li + 1) + m:
                                              2 * (li + 1) + m + 1], scale=1.0)
                        _mask_edges(nc, hcur, maske, li + 1)
                        hprev = hcur

                    # layer 4: C_out=8, bias only (no relu)
                    out5 = pp.tile([8, TSH], F32, tag="out5")
                    c0_, c1_ = _L_RANGES[4]
                    for (t0, t1_) in _ntiles(c0_, c1_):
                        n = t1_ - t0
                        ps = cp.tile([8, 512], F32, tag="cps5")
                        idx = 0
                        for ci in range(2):
                            for k in range(KSZ):
                                nc.tensor.matmul(
                                    ps[:, :n],
                                    w4[:, 8 * (5 * ci + k):8 * (5 * ci + k) + 8],
                                    hprev[ci][:, t0 + k - 2:t1_ + k - 2],
                                    start=(idx == 0), stop=(idx == 9))
                                idx += 1
                        nc.scalar.activation(
                            out5[:, t0 - OFF:t1_ - OFF], ps[:, :n],
                            AF.Identity, bias=ball[0:8, 8:9], scale=1.0)

                    nc.sync.dma_start(out=d_out[:], in_=out5[:])

    nc.compile()
    _cache["nc"] = nc
    _cache["names"] = names
    return nc, names


def _mask_edges(nc, hcur, maske, layer):
    o = _L_RANGES[layer][0]
    lw = OFF - o
    for m in range(2):
        nc.vector.tensor_tensor(hcur[m][:, o:OFF], hcur[m][:, o:OFF],
                                maske[:, o:OFF], ALU.mult)
        nc.vector.tensor_tensor(hcur[m][:, 2060:2060 + lw],
                                hcur[m][:, 2060:2060 + lw],
                                maske[:, 12:12 + lw], ALU.mult)


LAST_EXEC_NS = None


def kernel(X_c, y_c, t, log_l, conv_weights, conv_biases):
    global LAST_EXEC_NS
    X_c = np.asarray(X_c, dtype=np.float32)
    y_c = np.asarray(y_c, dtype=np.float32)
    t = np.asarray(t, dtype=np.float32)
    log_l = np.asarray(log_l, dtype=np.float32)
    Ws = [np.asarray(w, dtype=np.float32) for w in conv_weights]
    bs = [np.asarray(b, dtype=np.float32) for b in conv_biases]

    l = float(np.exp(log_l[0]))
    sx = (X_c[:, 0] / l).astype(np.float32)           # (2048,)
    st = (t[:, 0] / l).astype(np.float32)             # (16384,)

    st_pad = np.zeros(OFF + T_PTS + (W + 128), dtype=np.float32)
    st_pad[OFF:OFF + T_PTS] = st

    sx_pm = np.ascontiguousarray(sx.reshape(NCTX, P).T)   # (128, 16)
    sx_col = np.concatenate([sx_pm, -sx_pm], axis=1).astype(np.float32)

    phiT = np.zeros((P, 2 * NCTX), dtype=np.float16)
    for b in range(NCTX):
        phiT[:, 2 * b] = 1.0
        phiT[:, 2 * b + 1] = y_c[P * b:P * b + P].astype(np.float16)

    w0T = np.ascontiguousarray(
        Ws[0].transpose(2, 1, 0).reshape(2 * KSZ, CH)).astype(np.float16)

    wmT = []
    for li in range(3):
        Wl = Ws[li + 1]
        arr = np.zeros((P, 20 * P), dtype=np.float16)
        for m in range(2):
            for ci in range(2):
                for k in range(KSZ):
                    idx = 10 * m + 5 * ci + k
                    arr[:, P * idx:P * idx + P] = (
                        Wl[P * m:P * m + P, P * ci:P * ci + P, k].T
                        .astype(np.float16))
        wmT.append(arr)

    w4T = np.zeros((P, 80), dtype=np.float16)
    for ci in range(2):
        for k in range(KSZ):
            idx = 5 * ci + k
            w4T[:, 8 * idx:8 * idx + 8] = (
                Ws[4][:, P * ci:P * ci + P, k].T.astype(np.float16))

    b_all = np.zeros((P, 9), dtype=np.float32)
    for li in range(4):
        for m in range(2):
            b_all[:, 2 * li + m] = bs[li][P * m:P * m + P]
    b_all[0:8, 8] = bs[4]

    nc, nm = _build()

    in_maps = []
    for c in range(NCORES):
        s = c * TSH
        stb = np.ascontiguousarray(
            np.broadcast_to(st_pad[s:s + W][None, :], (P, W))).astype(np.float32)

        g = (s - OFF) + np.arange(W)          # global col of each buffer col
        valid = (g >= 0) & (g < T_PTS)
        maskT = np.ascontiguousarray(
            valid.astype(np.float32).reshape(P, 17))
        maske = np.zeros((P, 24), dtype=np.float16)
        maske[:, 0:12] = valid[0:12][None, :]
        maske[:, 12:24] = valid[2060:2072][None, :]

        in_maps.append({
            nm["stb"]: stb, nm["sx"]: sx_col, nm["phi"]: phiT,
            nm["w0"]: w0T, nm["w1"]: wmT[0], nm["w2"]: wmT[1],
            nm["w3"]: wmT[2], nm["w4"]: w4T, nm["b"]: b_all,
            nm["mt"]: maskT, nm["me"]: maske,
        })

    trace = os.environ.get("KERNEL_TRACE", "0") == "1"
    res = run_bass_kernel_spmd(nc, in_maps, list(range(NCORES)), trace=trace)
    LAST_EXEC_NS = res.exec_time_ns

    out = np.empty((T_PTS, 8), dtype=np.float32)
    for c in range(NCORES):
        out[c * TSH:(c + 1) * TSH, :] = res.results[c][nm["out"]].T
    return out


# revision 14
# speedup vs baseline: 1.0320x; 1.0052x over previous
"""ConvDeepSet Trainium2 kernel (8 NeuronCores, SPMD over the T/grid axis).

Math (per reference):
  l = exp(log_l);  sx = X_c/l;  st = t/l
  K[n,t] = exp(-0.5*|sx_n - st_t|)                 (x_dim == 1)
  E[c,t] = sum_n phi[n,c] * K[n,t],  phi = [1, y_c]
  h0 = [E0, E1/E0]  -> 5x Conv1d(ksize 5, SAME) with ReLU between
  out = h5.T  (T, 8)

Sharding: T axis split across 8 cores (2048 cols each). Context points and
CNN weights are replicated, so each core computes its T-slice's pooled
signal plus a 10-column receptive-field halo locally — no collectives.
Each core's working width is W=2176 (=128*17) covering global columns
[s-12, s+2164); the conv stack consumes 2 halo columns per layer.
Global SAME-padding is honored by masking columns outside [0, 16384).

Device dtypes: fp32 for the |sx-st| computation and all PSUM accumulation;
fp16 for the kernel matrix, pooled signal and conv weights/activations
(measured end-to-end error ~1e-3 scaled-max vs the fp32 reference).
"""
import os

import numpy as np

import concourse.bass as bass
import concourse.mybir as mybir
import concourse.tile as tile
from concourse import bacc
from concourse.bass_utils import run_bass_kernel_spmd

F32 = mybir.dt.float32
F16 = mybir.dt.float16
I32 = mybir.dt.int32
AF = mybir.ActivationFunctionType
ALU = mybir.AluOpType

NCORES = 8
P = 128
T_PTS = 16384
TSH = T_PTS // NCORES          # 2048 columns per core
W = 2176                       # working width = 128*17 (12-col halo + spare)
OFF = 12                       # buffer col 0 <-> global col s-12
N_C = 2048
NCTX = N_C // P                # 16 context blocks
KSZ = 5
CH = 256                       # mid-layer channels
WP = 2072                      # pooled columns actually consumed downstream

# conv output column ranges within the working buffer, per layer
_L_RANGES = [(4, 2068), (6, 2066), (8, 2064), (10, 2062), (12, 2060)]


def _ntiles(c0, c1, step=512):
    out = []
    c = c0
    while c < c1:
        out.append((c, min(c + step, c1)))
        c += step
    return out


_cache = {}


def _build():
    if "nc" in _cache:
        return _cache["nc"], _cache["names"]

    nc = bacc.Bacc(None, target_bir_lowering=False, debug=False)
    names = {}
    with tile.TileContext(nc) as tc:
        with tc.tile_pool(name="dram", bufs=1, space="DRAM") as dram:
            d_stb = dram.tile([P, W], F32, kind="ExternalInput")
            d_sx = dram.tile([P, 2 * NCTX], F32, kind="ExternalInput")
            d_phi = dram.tile([P, 2 * NCTX], F16, kind="ExternalInput")
            d_w0 = dram.tile([2 * KSZ, CH], F16, kind="ExternalInput")
            d_wm = [dram.tile([P, 20 * P], F16, kind="ExternalInput",
                                name=f"d_wm{i}") for i in range(3)]
            d_w4 = dram.tile([P, 80], F16, kind="ExternalInput")
            d_b = dram.tile([P, 9], F32, kind="ExternalInput")
            d_mt = dram.tile([P, 17], F32, kind="ExternalInput")
            d_me = dram.tile([P, 24], F16, kind="ExternalInput")
            d_out = dram.tile([8, TSH], F32, kind="ExternalOutput")
            d_scr32 = dram.tile([2, W], F32, kind="Internal")
            d_scr16 = dram.tile([2, W], F16, kind="Internal")
            names = dict(stb=d_stb.name, sx=d_sx.name, phi=d_phi.name,
                         w0=d_w0.name, w1=d_wm[0].name, w2=d_wm[1].name,
                         w3=d_wm[2].name, w4=d_w4.name, b=d_b.name,
                         mt=d_mt.name, me=d_me.name, out=d_out.name)

            with (
                tc.tile_pool(name="persist", bufs=1) as pp,
                tc.tile_pool(name="work", bufs=3) as wp,
                tc.tile_pool(name="kp", bufs=4) as kp,
            ):
                stb = pp.tile([P, W], F32, tag="stb")
                sx = pp.tile([P, 2 * NCTX], F32, tag="sx")
                phi = pp.tile([P, 2 * NCTX], F16, tag="phi")
                w0 = pp.tile([2 * KSZ, CH], F16, tag="w0")
                wm = [pp.tile([P, 20 * P], F16, tag=f"wm{i}",
                              name=f"wm{i}") for i in range(3)]
                w4 = pp.tile([P, 80], F16, tag="w4")
                ball = pp.tile([P, 9], F32, tag="ball")
                maskT = pp.tile([P, 17], F32, tag="maskT")
                maske = pp.tile([P, 24], F16, tag="maske")

                nc.sync.dma_start(out=stb[:], in_=d_stb[:])
                nc.sync.dma_start(out=sx[:], in_=d_sx[:])
                nc.sync.dma_start(out=phi[:], in_=d_phi[:])
                nc.sync.dma_start(out=maskT[:], in_=d_mt[:])
                nc.sync.dma_start(out=maske[:], in_=d_me[:])

                h0 = pp.tile([2, W], F16, tag="h0")
                ezT = pp.tile([P, 34], F32, tag="ezT")

                pool_tiles = _ntiles(0, WP)

                # ---- pooling phase: K tiles + phi^T @ K accumulation ----
                with tc.tile_pool(name="ezp", bufs=1, space="PSUM") as ezp:
                    ez = ezp.tile([2, W], F32, tag="ez")
                    ACT_ABS_BLOCKS = (6, 13)
                    for b in range(NCTX):
                        da = wp.tile([P, WP], F32, tag="da")
                        if b in ACT_ABS_BLOCKS:
                            # |st - sx| in one ScalarE op (bias = -sx)
                            nc.scalar.activation(
                                da[:], stb[:, 0:WP], AF.Abs,
                                bias=sx[:, NCTX + b:NCTX + b + 1], scale=1.0)
                        else:
                            t1 = wp.tile([P, WP], F32, tag="t1")
                            nc.vector.tensor_scalar(
                                t1[:], stb[:, 0:WP], sx[:, b:b + 1], None,
                                ALU.subtract, ALU.bypass)
                            nc.vector.tensor_scalar(
                                da[:].bitcast(I32), t1[:].bitcast(I32),
                                0x7FFFFFFF, None, ALU.bitwise_and, ALU.bypass)
                        kb = kp.tile([P, WP], F16, tag="kb")
                        nc.scalar.activation(kb[:], da[:], AF.Exp,
                                             bias=0.0, scale=-0.5)
                        for (c0, c1) in pool_tiles:
                            nc.tensor.matmul(
                                ez[:, c0:c1], phi[:, 2 * b:2 * b + 2],
                                kb[:, c0:c1],
                                start=(b == 0), stop=(b == NCTX - 1))

                    # ---- normalization: transpose rows via DRAM, divide, mask ----
                    ezrow = pp.tile([2, W], F32, tag="ezrow")
                    nc.scalar.copy(ezrow[:, 0:W // 2], ez[:, 0:W // 2])
                    nc.vector.tensor_copy(ezrow[:, W // 2:], ez[:, W // 2:])
                    nc.sync.dma_start(out=d_scr32[0:1, :], in_=ezrow[0:1, :])
                    nc.scalar.dma_start(out=d_scr32[1:2, :], in_=ezrow[1:2, :])
                    for r, eng in ((0, nc.sync), (1, nc.scalar)):
                        eng.dma_start(
                            out=ezT[:, 17 * r:17 * r + 17],
                            in_=d_scr32[r:r + 1, :].rearrange(
                                "o (p j) -> (o p) j", p=P))

                rec = pp.tile([P, 17], F32, tag="rec")
                nc.vector.reciprocal(rec[:], ezT[:, 0:17])
                val1 = pp.tile([P, 17], F32, tag="val1")
                nc.vector.tensor_tensor(val1[:], ezT[:, 17:34], rec[:],
                                        ALU.mult)
                den_h = pp.tile([P, 17], F16, tag="den_h")
                nc.vector.tensor_tensor(den_h[:], ezT[:, 0:17], maskT[:],
                                        ALU.mult)
                val_h = pp.tile([P, 17], F16, tag="val_h")
                nc.vector.tensor_tensor(val_h[:], val1[:], maskT[:], ALU.mult)
                for r, src, eng in ((0, den_h, nc.sync), (1, val_h, nc.scalar)):
                    eng.dma_start(
                        out=d_scr16[r:r + 1, :].rearrange(
                            "o (p j) -> (o p) j", p=P),
                        in_=src[:])
                nc.sync.dma_start(out=h0[:], in_=d_scr16[:])

                # PE warm-keeper: dummy matmuls bridging the norm-chain
                # window so HAM stays at K=8/8 into the conv phase
                with tc.tile_pool(name="wmp", bufs=1, space="PSUM") as wmp:
                    wps = wmp.tile([2, 512], F32, tag="wps")
                    for i in range(26):
                        nc.tensor.matmul(wps[:], phi[:, 0:2], kb[:, 0:512],
                                         start=(i == 0), stop=(i == 25))

                # weight DMAs: issued late (separate queue; not on the
                # critical stb path — they overlap the pooling phase)
                nc.gpsimd.dma_start(out=w0[:], in_=d_w0[:])
                for i in range(3):
                    nc.gpsimd.dma_start(out=wm[i][:], in_=d_wm[i][:])
                nc.gpsimd.dma_start(out=w4[:], in_=d_w4[:])
                nc.gpsimd.dma_start(out=ball[:], in_=d_b[:])

                # ---- conv stack ----
                hprev = None
                with tc.tile_pool(name="cp", bufs=4, space="PSUM") as cp:
                    # layer 0: im2col rows (k, ci) -> one K=10 matmul/tile
                    hc = pp.tile([2 * KSZ, W], F16, tag="hc")
                    for k in range(KSZ):
                        eng = nc.sync if k % 2 == 0 else nc.scalar
                        eng.dma_start(out=hc[2 * k:2 * k + 2, 2:W - 2],
                                      in_=h0[0:2, k:W - 4 + k])
                    c0_, c1_ = _L_RANGES[0]
                    hcur = [pp.tile([P, W], F16, tag="h1_0", name="h1_0"),
                            pp.tile([P, W], F16, tag="h1_1", name="h1_1")]
                    for (t0, t1_) in _ntiles(c0_, c1_):
                        n = t1_ - t0
                        for m in range(2):
                            ps = cp.tile([P, 512], F32, tag="cps")
                            nc.tensor.matmul(
                                ps[:, :n], w0[:, P * m:P * m + P],
                                hc[:, t0:t1_], start=True, stop=True)
                            nc.scalar.activation(
                                hcur[m][:, t0:t1_], ps[:, :n], AF.Relu,
                                bias=ball[:, 0 + m:1 + m], scale=1.0)
                    _mask_edges(nc, hcur, maske, 0)
                    hprev = hcur

                    # layers 1..3: C 256->256
                    for li in range(3):
                        c0_, c1_ = _L_RANGES[li + 1]
                        hcur = [
                            pp.tile([P, W], F16, tag=f"h{li+2}_0",
                                    name=f"h{li+2}_0"),
                            pp.tile([P, W], F16, tag=f"h{li+2}_1",
                                    name=f"h{li+2}_1")]
                        for (t0, t1_) in _ntiles(c0_, c1_):
                            n = t1_ - t0
                            for m in range(2):
                                ps = cp.tile([P, 512], F32, tag="cps")
                                idx = 0
                                for ci in range(2):
                                    for k in range(KSZ):
                                        nc.tensor.matmul(
                                            ps[:, :n],
                                            wm[li][:, P * (10 * m + 5 * ci + k):
                                                   P * (10 * m + 5 * ci + k) + P],
                                            hprev[ci][:, t0 + k - 2:t1_ + k - 2],
                                            start=(idx == 0), stop=(idx == 9))
                                        idx += 1
                                nc.scalar.activation(
                                    hcur[m][:, t0:t1_], ps[:, :n], AF.Relu,
                                    bias=ball[:, 2 * (li + 1) + m:
                                              2 * (li + 1) + m + 1], scale=1.0)
                        _mask_edges(nc, hcur, maske, li + 1)
                        hprev = hcur

                    # layer 4: C_out=8, bias only (no relu)
                    out5 = pp.tile([8, TSH], F32, tag="out5")
                    c0_, c1_ = _L_RANGES[4]
                    for (t0, t1_) in _ntiles(c0_, c1_):
                        n = t1_ - t0
                        ps = cp.tile([8, 512], F32, tag="cps5")
                        idx = 0
                        for ci in range(2):
                            for k in range(KSZ):
                                nc.tensor.matmul(
                                    ps[:, :n],
                                    w4[:, 8 * (5 * ci + k):8 * (5 * ci + k) + 8],
                                    hprev[ci][:, t0 + k - 2:t1_ + k - 2],
                                    start=(idx == 0), stop=(idx == 9))
                                idx += 1
                        nc.scalar.activation(
                            out5[:, t0 - OFF:t1_ - OFF], ps[:, :n],
                            AF.Identity, bias=ball[0:8, 8:9], scale=1.0)

                    nc.sync.dma_start(out=d_out[:], in_=out5[:])

    nc.compile()
    _cache["nc"] = nc
    _cache["names"] = names
    return nc, names


def _mask_edges(nc, hcur, maske, layer):
    o = _L_RANGES[layer][0]
    lw = OFF - o
    for m in range(2):
        nc.vector.tensor_tensor(hcur[m][:, o:OFF], hcur[m][:, o:OFF],
                                maske[:, o:OFF], ALU.mult)
        nc.vector.tensor_tensor(hcur[m][:, 2060:2060 + lw],
                                hcur[m][:, 2060:2060 + lw],
                                maske[:, 12:12 + lw], ALU.mult)


LAST_EXEC_NS = None


def kernel(X_c, y_c, t, log_l, conv_weights, conv_biases):
    global LAST_EXEC_NS
    X_c = np.asarray(X_c, dtype=np.float32)
    y_c = np.asarray(y_c, dtype=np.float32)
    t = np.asarray(t, dtype=np.float32)
    log_l = np.asarray(log_l, dtype=np.float32)
    Ws = [np.asarray(w, dtype=np.float32) for w in conv_weights]
    bs = [np.asarray(b, dtype=np.float32) for b in conv_biases]

    l = float(np.exp(log_l[0]))
    sx = (X_c[:, 0] / l).astype(np.float32)           # (2048,)
    st = (t[:, 0] / l).astype(np.float32)             # (16384,)

    st_pad = np.zeros(OFF + T_PTS + (W + 128), dtype=np.float32)
    st_pad[OFF:OFF + T_PTS] = st

    sx_pm = np.ascontiguousarray(sx.reshape(NCTX, P).T)   # (128, 16)
    sx_col = np.concatenate([sx_pm, -sx_pm], axis=1).astype(np.float32)

    phiT = np.zeros((P, 2 * NCTX), dtype=np.float16)
    for b in range(NCTX):
        phiT[:, 2 * b] = 1.0
        phiT[:, 2 * b + 1] = y_c[P * b:P * b + P].astype(np.float16)

    w0T = np.ascontiguousarray(
        Ws[0].transpose(2, 1, 0).reshape(2 * KSZ, CH)).astype(np.float16)

    wmT = []
    for li in range(3):
        Wl = Ws[li + 1]
        arr = np.zeros((P, 20 * P), dtype=np.float16)
        for m in range(2):
            for ci in range(2):
                for k in range(KSZ):
                    idx = 10 * m + 5 * ci + k
                    arr[:, P * idx:P * idx + P] = (
                        Wl[P * m:P * m + P, P * ci:P * ci + P, k].T
                        .astype(np.float16))
        wmT.append(arr)

    w4T = np.zeros((P, 80), dtype=np.float16)
    for ci in range(2):
        for k in range(KSZ):
            idx = 5 * ci + k
            w4T[:, 8 * idx:8 * idx + 8] = (
                Ws[4][:, P * ci:P * ci + P, k].T.astype(np.float16))

    b_all = np.zeros((P, 9), dtype=np.float32)
    for li in range(4):
        for m in range(2):
            b_all[:, 2 * li + m] = bs[li][P * m:P * m + P]
    b_all[0:8, 8] = bs[4]

    nc, nm = _build()

    in_maps = []
    for c in range(NCORES):
        s = c * TSH
        stb = np.ascontiguousarray(
            np.broadcast_to(st_pad[s:s + W][None, :], (P, W))).astype(np.float32)

        g = (s - OFF) + np.arange(W)          # global col of each buffer col
        valid = (g >= 0) & (g < T_PTS)
        maskT = np.ascontiguousarray(
            valid.astype(np.float32).reshape(P, 17))
        maske = np.zeros((P, 24), dtype=np.float16)
        maske[:, 0:12] = valid[0:12][None, :]
        maske[:, 12:24] = valid[2060:2072][None, :]

        in_maps.append({
            nm["stb"]: stb, nm["sx"]: sx_col, nm["phi"]: phiT,
            nm["w0"]: w0T, nm["w1"]: wmT[0], nm["w2"]: wmT[1],
            nm["w3"]: wmT[2], nm["w4"]: w4T, nm["b"]: b_all,
            nm["mt"]: maskT, nm["me"]: maske,
        })

    trace = os.environ.get("KERNEL_TRACE", "0") == "1"
    res = run_bass_kernel_spmd(nc, in_maps, list(range(NCORES)), trace=trace)
    LAST_EXEC_NS = res.exec_time_ns

    out = np.empty((T_PTS, 8), dtype=np.float32)
    for c in range(NCORES):
        out[c * TSH:(c + 1) * TSH, :] = res.results[c][nm["out"]].T
    return out
